# Initial kernel scaffold
#
"""Trainium2 Bass kernel for nn_MessagePassingLayer (GNN message passing).

reference semantics (per batch b):
  cm  = adj[b].T @ ps[b]                  # [C, H] channel aggregation
  ncs = GRUCell(x=cs[b], h=cm)            # new channel states
  pm  = adj[b] @ ncs                      # [P, H] path aggregation
  nps = GRUCell(x=ps[b], h=pm)            # new path states
  returns (nps, ncs)

Sharding: data-parallel over batch, 2 batches per core x 8 cores.

Per-core design (memory-regime: adj is 16MB/batch and is the traffic):
  - adj[b] is DMA'd from HBM ONCE (f32->bf16 cast in flight, SWDGE),
    consumed as p-slabs [128, C] by einsum1 (moving operand), and
    PE-transposed tile-by-tile into a persistent AT [c_lo, j, p] bf16
    for einsum2. The reference reads adj twice; we read once.
  - einsum1 computed transposed: cmT[h, c] += ps_tile.T-form matmuls
    (lhsT = ps tile [p,H], rhs = A slab) accumulating in PSUM f32.
  - GRU gates feature-major: giT/ghT [3H, n] = w^T-form matmuls with
    rhs = xT/hT [H, n]; biases are per-partition ACT bias APs.
  - einsum2 transposed: pmT[h, p] += (lhsT = ncs tile [c,H],
    rhs = AT slab [c, p]) accumulating in PSUM f32.
  - outputs packed on-chip to [q, (g l h)] so each partition's HBM run
    is 512B (DMA line-rate), via stride-4 PE transposes.
"""

import numpy as np

import concourse.bass as bass
import concourse.tile as tile
from concourse import bacc, masks, mybir
from concourse.bass_utils import run_bass_kernel_spmd

F32 = mybir.dt.float32
# 2-byte compute dtype: fp16 (10-bit mantissa) — adj in [0,1), states O(1),
# messages O(30): all comfortably in fp16 range, 4x less rounding than bf16.
BF16 = mybir.dt.float16

B, P, C, H = 16, 2048, 2048, 32
G = 3 * H  # 96
NCORES = 8
BPC = B // NCORES  # batches per core
PB = 128  # partition block
NP = P // PB  # 16 p-chunks
NC = C // PB  # 16 c-chunks
NKC = 512  # matmul moving chunk (one PSUM f32 bank)


def _gru(tc, pool, ps_misc, wT_ih, wT_hh, xT, hT, b_rz, bias_n,
         st_pool, out_tag, dt_b=BF16, g_engine="act"):
    """Feature-major GRUCell -> SBUF [H, N] tile (dtype dt).

    Per 512-col chunk, one PSUM tile [128, 512] f32 holds:
      rows 0:64   = i_rz + h_rz   (two accumulating matmuls)
      rows 64:96  = i_n
      rows 96:128 = h_n
    r/z/gin/g are produced by single-input ACT/DVE ops reading PSUM
    (bias folded, base partition moved to 0), then 5 full-width
    TensorTensor ops combine. No gates SBUF tile, no separate adds.
    """
    nc = tc.nc
    AF = mybir.ActivationFunctionType
    N = xT.shape[-1]
    out = st_pool.tile([H, N], dt_b, tag=out_tag)
    r = pool.tile([H, N], BF16, tag="gru_r")
    z = pool.tile([H, N], BF16, tag="gru_z")
    g = pool.tile([H, N], BF16, tag="gru_g")
    gin = pool.tile([H, N], BF16, tag="gru_gin")
    for q in range(N // NKC):
        gp = ps_misc.tile([PB, NKC], F32, tag="sm")
        sl = slice(q * NKC, (q + 1) * NKC)
        nc.tensor.matmul(gp[0 : 2 * H, :], wT_ih[:, 0 : 2 * H], xT[:, sl],
                         start=True, stop=False)
        nc.tensor.matmul(gp[0 : 2 * H, :], wT_hh[:, 0 : 2 * H], hT[:, sl],
                         start=False, stop=True)
        nc.tensor.matmul(gp[2 * H : G, :], wT_ih[:, 2 * H : G], xT[:, sl],
                         start=True, stop=True)
        nc.tensor.matmul(gp[G : G + H, :], wT_hh[:, 2 * H : G], hT[:, sl],
                         start=True, stop=True, tile_position=(0, 96))
        nc.scalar.activation(r[:, sl], gp[0:H, :], AF.Sigmoid, bias=b_rz[0:H, :])
        nc.scalar.activation(z[:, sl], gp[H : 2 * H, :], AF.Sigmoid,
                             bias=b_rz[H : 2 * H, :])
        if g_engine == "act":
            nc.scalar.activation(g[:, sl], gp[G : G + H, :], AF.Identity,
                                 bias=bias_n[G : G + H, :])
        else:
            nc.vector.tensor_scalar_add(g[:, sl], gp[G : G + H, :],
                                        bias_n[G : G + H, :])
        nc.vector.tensor_scalar_add(gin[:, sl], gp[2 * H : G, :],
                                    bias_n[2 * H : G, :])
    t1 = pool.tile([H, N], BF16, tag="gru_t1")
    nc.vector.tensor_mul(t1[:], r[:], g[:])
    npre = pool.tile([H, N], BF16, tag="gru_g")
    nc.vector.tensor_add(npre[:], gin[:], t1[:])
    ng = pool.tile([H, N], dt_b, tag="gru_t1")
    nc.scalar.activation(ng[:], npre[:], AF.Tanh)
    d = pool.tile([H, N], dt_b, tag="gru_g")
    nc.vector.tensor_sub(d[:], hT, ng[:])
    zd = pool.tile([H, N], dt_b, tag="gru_gin")
    nc.vector.tensor_mul(zd[:], z[:], d[:])
    nc.vector.tensor_add(out[:], ng[:], zd[:])
    return out


def build_nc(debug_outputs=False):
    nc = bacc.Bacc("TRN2", target_bir_lowering=False, debug=False,
                   num_devices=NCORES)

    adj = nc.dram_tensor("adj", [BPC, P, C], F32, kind="ExternalInput")
    ps = nc.dram_tensor("ps", [BPC, P, H], F32, kind="ExternalInput")
    cs = nc.dram_tensor("cs", [BPC, C, H], F32, kind="ExternalInput")
    w_ih_c = nc.dram_tensor("w_ih_c", [G, H], F32, kind="ExternalInput")
    w_hh_c = nc.dram_tensor("w_hh_c", [G, H], F32, kind="ExternalInput")
    w_ih_p = nc.dram_tensor("w_ih_p", [G, H], F32, kind="ExternalInput")
    w_hh_p = nc.dram_tensor("w_hh_p", [G, H], F32, kind="ExternalInput")
    b_ih_c = nc.dram_tensor("b_ih_c", [G, 1], F32, kind="ExternalInput")
    b_hh_c = nc.dram_tensor("b_hh_c", [G, 1], F32, kind="ExternalInput")
    b_ih_p = nc.dram_tensor("b_ih_p", [G, 1], F32, kind="ExternalInput")
    b_hh_p = nc.dram_tensor("b_hh_p", [G, 1], F32, kind="ExternalInput")
    out_np = nc.dram_tensor("new_path", [BPC, P, H], F32, kind="ExternalOutput")
    out_nc = nc.dram_tensor("new_channel", [BPC, C, H], F32, kind="ExternalOutput")
    dbg = {}
    if debug_outputs:
        dbg["cmT"] = nc.dram_tensor("dbg_cmT", [BPC, H, C], F32, kind="ExternalOutput")
        dbg["pmT"] = nc.dram_tensor("dbg_pmT", [BPC, H, P], F32, kind="ExternalOutput")
        dbg["ncsT"] = nc.dram_tensor("dbg_ncsT", [BPC, H, C], F32, kind="ExternalOutput")

    with tile.TileContext(nc) as tc:
        _body(tc, adj, ps, cs,
              (w_ih_c, w_hh_c, b_ih_c, b_hh_c),
              (w_ih_p, w_hh_p, b_ih_p, b_hh_p),
              out_np, out_nc, dbg)
    nc.finalize()
    return nc


def _body(tc, adj, ps, cs, wc, wp, out_np, out_nc, dbg):
    nc = tc.nc
    from contextlib import ExitStack

    ctx = ExitStack()
    with ctx:
        const = ctx.enter_context(tc.tile_pool(name="const", bufs=1))
        a_pool = ctx.enter_context(tc.tile_pool(name="a_slabs", bufs=4))
        at_pool = ctx.enter_context(tc.tile_pool(name="at", bufs=2))
        st_pool = ctx.enter_context(tc.tile_pool(name="states", bufs=1))
        gru_pool = ctx.enter_context(tc.tile_pool(name="gru", bufs=1))
        out_pool = ctx.enter_context(tc.tile_pool(name="outs", bufs=1))
        # PSUM banks: ps_mm 4 + ps_tp 2 + ps_misc 2 = 8
        ps_mm = ctx.enter_context(tc.tile_pool(name="ps_mm", bufs=1, space="PSUM"))
        ps_tp = ctx.enter_context(tc.tile_pool(name="ps_tp", bufs=2, space="PSUM"))
        ps_misc = ctx.enter_context(tc.tile_pool(name="ps_misc", bufs=2, space="PSUM"))

        ident = const.tile([PB, PB], BF16)
        masks.make_identity(nc, ident[:])
        ident_f = const.tile([PB, PB], F32)
        masks.make_identity(nc, ident_f[:])
        idents = {BF16: ident, F32: ident_f}

        # ---- weights: load [G, H], transpose to [H, G] via identity matmul ----
        # hhp stays f32: it multiplies path_msg (~1e5 scale) where the
        # z-gate argument needs small absolute error.
        wT = {}
        for name, wdram, wdt in (("ihc", wc[0], BF16), ("hhc", wc[1], BF16),
                                 ("ihp", wp[0], BF16), ("hhp", wp[1], F32)):
            w_ld = const.tile([G, H], wdt, tag=f"w_{name}")
            nc.gpsimd.dma_start(w_ld[:], wdram[:, :])
            wt_ps = ps_misc.tile([H, G], F32, tag="sm")
            nc.tensor.matmul(wt_ps[:], w_ld[:], idents[wdt][0:G, 0:G],
                             start=True, stop=True)
            wt = const.tile([H, G], wdt, tag=f"wT_{name}")
            nc.scalar.copy(wt[:], wt_ps[:])
            wT[name] = wt

        # ---- biases ----
        # bias_n[64:96] = b_ih_n, bias_n[96:128] = b_hh_n  (partition-aligned
        # with the PSUM gate layout: rows 64:96 = i_n, 96:128 = h_n)
        bias = {}
        for s, (bih, bhh) in (("c", (wc[2], wc[3])), ("p", (wp[2], wp[3]))):
            bn = const.tile([PB, 1], F32, tag=f"bn_{s}")
            nc.sync.dma_start(bn[2 * H : G, :], bih[2 * H : G, :])
            nc.sync.dma_start(bn[G : G + H, :], bhh[2 * H : G, :])
            ihrz = const.tile([2 * H, 1], F32, tag=f"bi_{s}")
            nc.sync.dma_start(ihrz[:], bih[0 : 2 * H, :])
            hhrz = const.tile([2 * H, 1], F32, tag=f"bh_{s}")
            nc.sync.dma_start(hhrz[:], bhh[0 : 2 * H, :])
            brz = const.tile([2 * H, 1], F32, tag=f"brz_{s}")
            nc.vector.tensor_add(brz[:], ihrz[:], hhrz[:])
            bias[s] = (brz, bn)

        for b in range(BPC):
            # ---- states: natural tiles (cast-DMA) + feature-major via PE ----
            ps_nat = st_pool.tile([PB, NP, H], BF16, tag="ps_nat")
            nc.gpsimd.dma_start(
                ps_nat[:], ps[b].rearrange("(i p) h -> p i h", p=PB))
            cs_nat = st_pool.tile([PB, NC, H], BF16, tag="cs_nat")
            nc.gpsimd.dma_start(
                cs_nat[:], cs[b].rearrange("(i p) h -> p i h", p=PB))

            sT = {}
            for nm, nat, nch in (("psT", ps_nat, NP), ("csT", cs_nat, NC)):
                dst = st_pool.tile([H, nch * PB], BF16, tag=nm)
                for quad in range(nch // 4):
                    tp = ps_misc.tile([H, 4, PB], F32, tag="sm")
                    for k in range(4):
                        nc.tensor.matmul(tp[:, k, :], nat[:, quad * 4 + k, :],
                                         ident[:, :], start=True, stop=True)
                    nc.scalar.copy(
                        dst[:, quad * 4 * PB : (quad + 1) * 4 * PB], tp[:])
                sT[nm] = dst

            # ---- stream A: einsum1 (cmT) + transposes into AT ----
            # transpose = regular identity matmul (out = slab_tile.T @ I):
            # pipelines at ~81ns and keeps the PE HAM clock warm, unlike
            # transpose-mode.
            at = at_pool.tile([PB, NC, P], BF16, tag="at")
            # col-packed 4x einsum: group g computes cm^T[:, 512g:512(g+1)]
            # on PE column-group g -> PSUM partitions 32g, bank g.
            cmT = ps_mm.tile([PB, 4, NKC], F32, tag="mm")
            for i in range(NP):
                slab = a_pool.tile([PB, C], BF16, tag="a")
                nc.gpsimd.dma_start(slab[:], adj[b, i * PB : (i + 1) * PB, :])
                for n in range(C // NKC):
                    nc.tensor.matmul(
                        cmT[n * H : (n + 1) * H, n, :],
                        ps_nat[:, i, :],
                        slab[:, n * NKC : (n + 1) * NKC],
                        start=(i == 0), stop=(i == NP - 1),
                        tile_position=(0, n * H),
                    )
                for quad in range(NC // 4):
                    tp = ps_tp.tile([PB, 4, PB], F32, tag="tp")
                    for k in range(4):
                        j = quad * 4 + k
                        # transpose tile j as 4 col-strips: weights are
                        # [128, 32] (cheap ldweights), strips run
                        # concurrently on distinct PE column-groups and
                        # stack vertically into the transposed tile.
                        for s in range(4):
                            nc.tensor.matmul(
                                tp[s * H : (s + 1) * H, k, :],
                                slab[:, j * PB + s * H : j * PB + (s + 1) * H],
                                ident[:, :], start=True, stop=True,
                                tile_position=(0, s * H),
                            )
                    ev = at[:, quad * 4 : (quad + 1) * 4, i * PB : (i + 1) * PB]
                    if (i + quad) % 2 == 0:
                        nc.scalar.copy(ev, tp[:])
                    else:
                        nc.vector.tensor_copy(ev, tp[:])

            # ---- GRU-c ----
            cmT_s = st_pool.tile([H, C], BF16, tag="hback")
            for n in range(4):
                nc.scalar.copy(cmT_s[:, n * NKC : (n + 1) * NKC],
                               cmT[n * H : (n + 1) * H, n, :])
            if "cmT" in dbg:
                nc.gpsimd.dma_start(dbg["cmT"][b], cmT_s[:])

            ncsT = _gru(tc, gru_pool, ps_misc, wT["ihc"], wT["hhc"],
                        sT["csT"], cmT_s, bias["c"][0], bias["c"][1],
                        st_pool, "mid", dt_b=BF16, g_engine="act")

            if "ncsT" in dbg:
                nc.gpsimd.dma_start(dbg["ncsT"][b], ncsT[:])

            # ncs natural tiles [c_lo, j, H] for einsum2 lhsT
            ncs_nat = st_pool.tile([PB, NC, H], BF16, tag="ncs_nat")
            tpn = ps_misc.tile([PB, NC, H], F32, tag="sm")
            for j in range(NC):
                nc.tensor.matmul(tpn[:, j, :], ncsT[:, j * PB : (j + 1) * PB],
                                 ident[0:H, 0:H], start=True, stop=True)
            nc.vector.tensor_copy(ncs_nat[:], tpn[:])

            # packed output new_channel
            _pack_out(tc, ncsT, out_nc[b], ps_misc, out_pool, idents)

            # ---- einsum2: pmT ----
            pmT = ps_mm.tile([PB, 4, NKC], F32, tag="mm")
            for j in range(NC):
                for n in range(P // NKC):
                    nc.tensor.matmul(
                        pmT[n * H : (n + 1) * H, n, :],
                        ncs_nat[:, j, :],
                        at[:, j, n * NKC : (n + 1) * NKC],
                        start=(j == 0), stop=(j == NC - 1),
                        tile_position=(0, n * H),
                    )
            pmT_s = st_pool.tile([H, P], F32, tag="mid")
            for n in range(4):
                nc.scalar.copy(pmT_s[:, n * NKC : (n + 1) * NKC],
                               pmT[n * H : (n + 1) * H, n, :])
            if "pmT" in dbg:
                nc.sync.dma_start(dbg["pmT"][b], pmT_s[:])

            # ---- GRU-p (f32 h-side: path_msg ~1e5 needs f32) ----
            npT = _gru(tc, gru_pool, ps_misc, wT["ihp"], wT["hhp"],
                       sT["psT"], pmT_s, bias["p"][0], bias["p"][1],
                       st_pool, "hback", dt_b=F32, g_engine="dve")

            _pack_out(tc, npT, out_np[b], ps_misc, out_pool, idents)


def _pack_out(tc, srcT, dram_b, ps_misc, out_pool, idents):
    """srcT [H, N] -> HBM [N, H] f32 with 512B-per-partition runs.

    Packed SBUF layout [q, g, l, h]: row index n = 512*g + 4*q + l.
    """
    nc = tc.nc
    dt = srcT.dtype
    N = srcT.shape[-1]
    NG = N // 512
    src_r = srcT.rearrange("h (g q l) -> h g q l", g=NG, l=4)
    sb = out_pool.tile([PB, NG, 4, H], F32, tag="opack")
    for g in range(NG):
        pk = ps_misc.tile([PB, 4, H], F32, tag="sm")
        for l in range(4):
            nc.tensor.matmul(pk[:, l, :], src_r[:, g, :, l],
                             idents[dt][0:H, 0:H], start=True, stop=True)
        nc.scalar.copy(sb[:, g, :, :], pk[:])
    nc.sync.dma_start(
        dram_b.rearrange("(g q l) h -> q g l h", q=PB, l=4), sb[:])


# ---------------------------------------------------------------------------
# host-side entry
# ---------------------------------------------------------------------------

_NC_CACHE = {}


def _get_nc(debug_outputs=False):
    key = bool(debug_outputs)
    if key not in _NC_CACHE:
        _NC_CACHE[key] = build_nc(debug_outputs=key)
    return _NC_CACHE[key]


def kernel(path_states, channel_states, adj_matrix,
           w_ih_c, w_hh_c, b_ih_c, b_hh_c,
           w_ih_p, w_hh_p, b_ih_p, b_hh_p,
           _debug=False, _trace=False):
    nc = _get_nc(debug_outputs=_debug)
    f32 = np.float32
    in_maps = []
    for k in range(NCORES):
        s = slice(k * BPC, (k + 1) * BPC)
        in_maps.append({
            "adj": np.ascontiguousarray(adj_matrix[s], f32),
            "ps": np.ascontiguousarray(path_states[s], f32),
            "cs": np.ascontiguousarray(channel_states[s], f32),
            "w_ih_c": np.ascontiguousarray(w_ih_c, f32),
            "w_hh_c": np.ascontiguousarray(w_hh_c, f32),
            "w_ih_p": np.ascontiguousarray(w_ih_p, f32),
            "w_hh_p": np.ascontiguousarray(w_hh_p, f32),
            "b_ih_c": np.ascontiguousarray(b_ih_c, f32).reshape(G, 1),
            "b_hh_c": np.ascontiguousarray(b_hh_c, f32).reshape(G, 1),
            "b_ih_p": np.ascontiguousarray(b_ih_p, f32).reshape(G, 1),
            "b_hh_p": np.ascontiguousarray(b_hh_p, f32).reshape(G, 1),
        })
    res = run_bass_kernel_spmd(nc, in_maps, core_ids=list(range(NCORES)),
                               trace=_trace)
    new_path = np.concatenate([res.results[k]["new_path"] for k in range(NCORES)])
    new_channel = np.concatenate(
        [res.results[k]["new_channel"] for k in range(NCORES)])
    out = (new_path, new_channel)
    if _debug or _trace:
        return out, res
    return out



# revision 11
# speedup vs baseline: 1.1021x; 1.1021x over previous
"""Trainium2 Bass kernel for nn_MessagePassingLayer (GNN message passing).

reference semantics (per batch b):
  cm  = adj[b].T @ ps[b]                  # [C, H] channel aggregation
  ncs = GRUCell(x=cs[b], h=cm)            # new channel states
  pm  = adj[b] @ ncs                      # [P, H] path aggregation
  nps = GRUCell(x=ps[b], h=pm)            # new path states
  returns (nps, ncs)

Sharding: data-parallel over batch, 2 batches per core x 8 cores.

Per-core design (memory regime: adj is 16MB/batch and is the traffic):
  - adj[b] DMA'd once (f32->fp16 cast in flight), consumed as p-slabs
    [128, C]: einsum1 uses them as MOVING data (4 matmuls per slab into a
    single packed PSUM bank via tile_position column packing), and each
    [128,128] tile is PE-transposed ONCE (full-tile, not strips) into a
    persistent AT [c_lo, j, p] fp16 for einsum2.
  - everything feature-major lives PACKED as [128, 512]: row 32*q + h holds
    feature h of column chunk q (512 wide). einsum outputs land packed
    directly (tile_position col = 32*q), so GRU gate math runs full-width
    [128, 512] ops instead of [32, 2048].
  - GRU is gate-major: four PSUM banks (r, z, g_in, g_hn), each packed
    [128, 512], written by matmuls placed at array tile (32q, 32q) with
    weights replicated at 4 partition offsets. Gates + combines are 9
    full-width ops with biases folded via ACT bias / scalar_tensor_tensor.
  - batch tails (GRU-c, einsum2, GRU-p, packing) are software-pipelined
    into the next batch's slab-DMA window so PE never waits on DMA.
"""

import numpy as np

import concourse.bass as bass
import concourse.tile as tile
from concourse import bacc, masks, mybir
from concourse.bass_utils import run_bass_kernel_spmd

F32 = mybir.dt.float32
F32R = mybir.dt.float32r
# 2-byte compute dtype: fp16 (10-bit mantissa) — adj in [0,1), states O(1),
# messages O(1e3): all in fp16 range, 4x less rounding than bf16.
F16 = mybir.dt.float16

B, P, C, H = 16, 2048, 2048, 32
G = 3 * H  # 96
NCORES = 8
BPC = B // NCORES  # batches per core
PB = 128
NP = P // PB  # 16 p-chunks
NC = C // PB  # 16 c-chunks
NKC = 512  # packed chunk width
NQ = 4  # chunks per 2048

AT_TMODE = False  # PE transpose-mode for AT (fp16 PSUM); False = identity mm
HH_P_DT = F32  # dtype for GRU-p h-side matmuls (F32 | F32R)

AF = mybir.ActivationFunctionType
ALU = mybir.AluOpType


def build_nc(debug_outputs=False, n_devices=NCORES):
    nc = bacc.Bacc("TRN2", target_bir_lowering=False, debug=False,
                   num_devices=n_devices)

    adj = nc.dram_tensor("adj", [BPC, P, C], F32, kind="ExternalInput")
    ps = nc.dram_tensor("ps", [BPC, P, H], F32, kind="ExternalInput")
    cs = nc.dram_tensor("cs", [BPC, C, H], F32, kind="ExternalInput")
    # host-pretransposed weights [H, G]
    wt = {}
    for nm in ("ihc", "hhc", "ihp", "hhp"):
        wt[nm] = nc.dram_tensor(f"wt_{nm}", [H, G], F32, kind="ExternalInput")
    # host-prepacked per-gate biases, replicated over the 4 row groups:
    # [4, PB, 1] = (r, z, in, hn) x [128, 1]
    b4 = {}
    for s in ("c", "p"):
        b4[s] = nc.dram_tensor(f"b4_{s}", [4, PB, 1], F32, kind="ExternalInput")
    out_np = nc.dram_tensor("new_path", [BPC, P, H], F32, kind="ExternalOutput")
    out_nc = nc.dram_tensor("new_channel", [BPC, C, H], F32, kind="ExternalOutput")
    dbg = {}
    if debug_outputs:
        dbg["cmT"] = nc.dram_tensor("dbg_cmT", [BPC, H, C], F32, kind="ExternalOutput")
        dbg["pmT"] = nc.dram_tensor("dbg_pmT", [BPC, H, P], F32, kind="ExternalOutput")
        dbg["ncsT"] = nc.dram_tensor("dbg_ncsT", [BPC, H, C], F32, kind="ExternalOutput")

    with tile.TileContext(nc) as tc:
        _body(tc, adj, ps, cs, wt, b4, out_np, out_nc, dbg)
    nc.finalize()
    return nc


def _body(tc, adj, ps, cs, wt, b4, out_np, out_nc, dbg):
    nc = tc.nc
    from contextlib import ExitStack

    ctx = ExitStack()
    with ctx:
        const = ctx.enter_context(tc.tile_pool(name="const", bufs=1))
        a_pool = ctx.enter_context(tc.tile_pool(name="a_slabs", bufs=4))
        at_pool = ctx.enter_context(tc.tile_pool(name="at", bufs=2))
        st = ctx.enter_context(tc.tile_pool(name="states", bufs=2))
        gw = ctx.enter_context(tc.tile_pool(name="gru", bufs=1))
        outp = ctx.enter_context(tc.tile_pool(name="outs", bufs=2))
        # PSUM: cm 1 bank + at 2 + gates/misc 4 + pack 2*(0.25) <= 8
        ps_cm = ctx.enter_context(tc.tile_pool(name="ps_cm", bufs=1, space="PSUM"))
        ps_at = ctx.enter_context(tc.tile_pool(name="ps_at", bufs=2, space="PSUM"))
        ps_g = ctx.enter_context(tc.tile_pool(name="ps_g", bufs=4, space="PSUM"))
        ps_mx = ctx.enter_context(tc.tile_pool(name="ps_mx", bufs=1, space="PSUM"))

        ident = const.tile([PB, PB], F16)
        masks.make_identity(nc, ident[:])
        ident_f = const.tile([PB, PB], F32)
        masks.make_identity(nc, ident_f[:])

        # ---- weights: [H, G] host-pretransposed, replicated at 4 partition
        # offsets so gate-major matmuls can place tiles at array row 32q.
        wt4 = {}
        for nm, wdt in (("ihc", F16), ("hhc", F16), ("ihp", F16), ("hhp", HH_P_DT)):
            w4 = const.tile([PB, G], wdt, tag=f"w4_{nm}")
            for q in range(4):
                eng = nc.gpsimd if wdt == F16 else nc.sync
                eng.dma_start(w4[32 * q : 32 * q + 32, :], wt[nm][:, :])
            wt4[nm] = w4

        # ---- biases: [128, 4] (cols r, z, in, hn), one DMA each
        bias = {}
        for s in ("c", "p"):
            bt = const.tile([PB, 4], F32, tag=f"b4_{s}")
            nc.sync.dma_start(bt[:], b4[s].rearrange("g p o -> p (g o)"))
            bias[s] = bt

        state = [dict() for _ in range(BPC)]

        def emit_states_dma(b):
            d = state[b]
            d["ps_nat"] = st.tile([PB, NP, H], F16, tag="ps_nat", name="ps_nat")
            nc.gpsimd.dma_start(
                d["ps_nat"][:], ps[b].rearrange("(i p) h -> p i h", p=PB))
            d["cs_nat"] = st.tile([PB, NC, H], F16, tag="cs_nat", name="cs_nat")
            nc.gpsimd.dma_start(
                d["cs_nat"][:], cs[b].rearrange("(i p) h -> p i h", p=PB))

        def emit_states_pack(b):
            # nat [128, 16, 32] -> packed xT [128, 512] (row 32q+h = feature
            # h of column chunk q) via 16 full-tile transposes per state.
            d = state[b]
            for nm, src in (("xp", d["ps_nat"]), ("xc", d["cs_nat"])):
                tp = ps_g.tile([PB, NKC], F32, tag="g")
                for q in range(4):
                    for k in range(4):
                        nc.tensor.matmul(
                            tp[32 * q : 32 * q + 32, 128 * k : 128 * (k + 1)],
                            src[:, 4 * q + k, :], ident[:, :],
                            start=True, stop=True, tile_position=(0, 32 * q))
                xt = st.tile([PB, NKC], F16, tag=nm, name=nm)
                nc.scalar.copy(xt[:], tp[:])
                d[nm] = xt

        def emit_slab(b, i):
            d = state[b]
            slab = a_pool.tile([PB, C], F16, tag="a")
            nc.gpsimd.dma_start(slab[:], adj[b, i * PB : (i + 1) * PB, :])
            # einsum1: cmT packed [128, 512] in ONE bank; chunk n rows 32n.
            for n in range(NQ):
                nc.tensor.matmul(
                    d["cm_ps"][32 * n : 32 * n + 32, :],
                    d["ps_nat"][:, i, :],
                    slab[:, n * NKC : (n + 1) * NKC],
                    start=(i == 0), stop=(i == NP - 1),
                    tile_position=(0, 32 * n), skip_group_check=True)
            # AT: full-tile transposes of the 16 [128,128] tiles of this slab
            at = d["at"]
            if AT_TMODE:
                for g in range(2):
                    tp = ps_at.tile([PB, 8, PB], F16, tag="atp")
                    for k in range(8):
                        j = 8 * g + k
                        nc.tensor.transpose(
                            tp[:, k, :], slab[:, j * PB : (j + 1) * PB],
                            ident[:, :])
                    ev = at[:, 8 * g : 8 * g + 8, i * PB : (i + 1) * PB]
                    eng = (nc.vector.tensor_copy, nc.scalar.copy,
                           nc.vector.tensor_copy)[(2 * i + g) % 3]
                    eng(ev, tp[:])
            else:
                for g in range(4):
                    tp = ps_at.tile([PB, 4, PB], F32, tag="atp")
                    for k in range(4):
                        j = 4 * g + k
                        nc.tensor.matmul(
                            tp[:, k, :], slab[:, j * PB : (j + 1) * PB],
                            ident[:, :], start=True, stop=True,
                            tile_position=(0, 0))
                    ev = at[:, 4 * g : 4 * g + 4, i * PB : (i + 1) * PB]
                    eng = (nc.vector.tensor_copy, nc.scalar.copy,
                           nc.vector.tensor_copy)[(4 * i + g) % 3]
                    eng(ev, tp[:])

        def gru_mm_group(xt, ht, w4_ih, w4_hh, banks, gidx):
            # gate-major: bank[gate] packed [128, 512], chunk q at rows 32q,
            # weights tile at array position (32q, 32q).
            bank_r, bank_z, bank_gi, bank_gh = banks

            def mm(bank, w4, cols, rhs, q, start, stop):
                nc.tensor.matmul(
                    bank[32 * q : 32 * q + 32, :],
                    w4[32 * q : 32 * q + 32, cols],
                    rhs[32 * q : 32 * q + 32, :],
                    start=start, stop=stop,
                    tile_position=(32 * q, 32 * q))

            for q in range(4):
                if gidx == 0:
                    mm(bank_r, w4_ih, slice(0, 32), xt, q, True, False)
                    mm(bank_r, w4_hh, slice(0, 32), ht, q, False, True)
                elif gidx == 1:
                    mm(bank_z, w4_ih, slice(32, 64), xt, q, True, False)
                    mm(bank_z, w4_hh, slice(32, 64), ht, q, False, True)
                else:
                    mm(bank_gi, w4_ih, slice(64, 96), xt, q, True, True)
                    mm(bank_gh, w4_hh, slice(64, 96), ht, q, True, True)

        def gru_eltwise1(banks, bt, tmp_dt):
            bank_r, bank_z, bank_gi, bank_gh = banks
            r = gw.tile([PB, NKC], F16, tag="gr")
            nc.scalar.activation(r[:], bank_r[:], AF.Sigmoid, bias=bt[:, 0:1])
            z = gw.tile([PB, NKC], F16, tag="gz")
            nc.scalar.activation(z[:], bank_z[:], AF.Sigmoid, bias=bt[:, 1:2])
            t1 = gw.tile([PB, NKC], tmp_dt, tag="gt1")
            nc.vector.scalar_tensor_tensor(
                t1[:], bank_gh[:], bt[:, 3:4], r[:], ALU.add, ALU.mult)
            npre = gw.tile([PB, NKC], tmp_dt, tag="gnp")
            nc.vector.scalar_tensor_tensor(
                npre[:], bank_gi[:], bt[:, 2:3], t1[:], ALU.add, ALU.add)
            return z, npre

        def gru_eltwise2(z, npre, ht, out, dt):
            ng = gw.tile([PB, NKC], dt, tag="gng")
            nc.scalar.activation(ng[:], npre[:], AF.Tanh)
            d = gw.tile([PB, NKC], dt, tag="gd")
            nc.vector.tensor_sub(d[:], ht[:], ng[:])
            zd = gw.tile([PB, NKC], dt, tag="gzd")
            nc.vector.tensor_mul(zd[:], z[:], d[:])
            nc.vector.tensor_add(out[:], ng[:], zd[:])

        def pack_out(srcp, idp, dram_b, tag):
            # packed [128, 512] -> HBM [2048, 32] f32, 512B runs/partition.
            sb = outp.tile([PB, NQ, 4, H], F32, tag=tag)
            for Q in range(4):
                pk = ps_mx.tile([PB, 4, H], F32, tag="pk")
                sp = srcp[32 * Q : 32 * Q + 32, :].rearrange(
                    "h (m l) -> h m l", l=4)
                idq = idp[32 * Q : 32 * Q + 32, 32 * Q : 32 * Q + 32]
                for l in range(4):
                    nc.tensor.matmul(pk[:, l, :], sp[:, :, l], idq,
                                     start=True, stop=True,
                                     tile_position=(32 * Q, 0))
                eng = (nc.scalar.copy, nc.vector.tensor_copy)[Q % 2]
                eng(sb[:, Q, :, :], pk[:])
            nc.sync.dma_start(
                dram_b.rearrange("(g m l) h -> m g l h", m=PB, l=4), sb[:])

        def tail_gen(b):
            # Generator: each yield marks a weave point. Tiles are allocated
            # at emission time so pool-buffer rotation matches program order.
            d = state[b]
            banks = [ps_g.tile([PB, NKC], F32, tag="g", name="g") for _ in range(4)]
            for gidx in range(3):
                gru_mm_group(d["xc"], d["cmTp"], wt4["ihc"], wt4["hhc"],
                             banks, gidx)
                yield
            z, npre = gru_eltwise1(banks, bias["c"], F16)
            yield
            d["ncsp"] = st.tile([PB, NKC], F16, tag="ncsp", name="ncsp")
            gru_eltwise2(z, npre, d["cmTp"], d["ncsp"], F16)
            if "ncsT" in dbg:
                nc.sync.dma_start(
                    dbg["ncsT"][b].rearrange("h (q c) -> (q h) c", q=NQ),
                    d["ncsp"][:])
            yield
            tpn = ps_g.tile([PB, NC, H], F32, tag="g")
            for j in range(NC):
                Q, k = j // 4, j % 4
                nc.tensor.matmul(
                    tpn[:, j, :],
                    d["ncsp"][32 * Q : 32 * Q + 32, 128 * k : 128 * (k + 1)],
                    ident[32 * Q : 32 * Q + 32, 32 * Q : 32 * Q + 32],
                    start=True, stop=True, tile_position=(32 * Q, 0))
            d["ncs_nat"] = st.tile([PB, NC, H], F16, tag="ncs_nat", name="ncs_nat")
            nc.vector.tensor_copy(d["ncs_nat"][:], tpn[:])
            yield
            pack_out(d["ncsp"], ident, out_nc[b], "onc")
            yield
            pm_ps = ps_g.tile([PB, NKC], F32, tag="g")
            for j0 in range(0, NC, 4):
                for j in range(j0, j0 + 4):
                    for n in range(NQ):
                        nc.tensor.matmul(
                            pm_ps[32 * n : 32 * n + 32, :],
                            d["ncs_nat"][:, j, :],
                            d["at"][:, j, n * NKC : (n + 1) * NKC],
                            start=(j == 0), stop=(j == NC - 1),
                            tile_position=(0, 32 * n), skip_group_check=True)
                yield
            d["pmTp"] = st.tile([PB, NKC], HH_P_DT, tag="pmTp", name="pmTp")
            nc.scalar.copy(d["pmTp"][:], pm_ps[:])
            if "pmT" in dbg:
                nc.sync.dma_start(
                    dbg["pmT"][b].rearrange("h (q c) -> (q h) c", q=NQ),
                    d["pmTp"][:])
            yield
            banks_p = [ps_g.tile([PB, NKC], F32, tag="g", name="g") for _ in range(4)]
            for gidx in range(3):
                gru_mm_group(d["xp"], d["pmTp"], wt4["ihp"], wt4["hhp"],
                             banks_p, gidx)
                yield
            z, npre = gru_eltwise1(banks_p, bias["p"], F32)
            yield
            d["npp"] = st.tile([PB, NKC], F32, tag="npp", name="npp")
            gru_eltwise2(z, npre, d["pmTp"], d["npp"], F32)
            yield
            pack_out(d["npp"], ident_f, out_np[b], "onp")

        # ================= main schedule =================
        tail = iter(())

        def drain(n):
            for _ in range(n):
                next(tail, None)

        emit_states_dma(0)
        for b in range(BPC):
            d = state[b]
            emit_states_pack(b)
            d["at"] = at_pool.tile([PB, NC, P], F16, tag="at", name="at")
            d["cm_ps"] = ps_cm.tile([PB, NKC], F32, tag="cm", name="cm")
            for i in range(NP):
                emit_slab(b, i)
                if i == 7 and b + 1 < BPC:
                    emit_states_dma(b + 1)
                drain(2 if i < 6 else 1)
            for _ in tail:
                pass
            # extract packed cmT before next batch's einsum1 reuses the bank
            d["cmTp"] = st.tile([PB, NKC], F16, tag="cmTp", name="cmTp")
            nc.scalar.copy(d["cmTp"][:], d["cm_ps"][:])
            if "cmT" in dbg:
                nc.sync.dma_start(
                    dbg["cmT"][b].rearrange("h (q c) -> (q h) c", q=NQ),
                    d["cmTp"][:])
            tail = tail_gen(b)
        for _ in tail:
            pass


# ---------------------------------------------------------------------------
# host-side entry
# ---------------------------------------------------------------------------

_NC_CACHE = {}


def _get_nc(debug_outputs=False):
    key = bool(debug_outputs)
    if key not in _NC_CACHE:
        _NC_CACHE[key] = build_nc(debug_outputs=key)
    return _NC_CACHE[key]


def _b4(b_ih, b_hh):
    f32 = np.float32
    b_ih = np.asarray(b_ih, f32).reshape(G)
    b_hh = np.asarray(b_hh, f32).reshape(G)
    r = np.tile(b_ih[0:32] + b_hh[0:32], 4)
    z = np.tile(b_ih[32:64] + b_hh[32:64], 4)
    bi = np.tile(b_ih[64:96], 4)
    bn = np.tile(b_hh[64:96], 4)
    return np.stack([r, z, bi, bn]).reshape(4, PB, 1)


def kernel(path_states, channel_states, adj_matrix,
           w_ih_c, w_hh_c, b_ih_c, b_hh_c,
           w_ih_p, w_hh_p, b_ih_p, b_hh_p,
           _debug=False, _trace=False):
    nc = _get_nc(debug_outputs=_debug)
    f32 = np.float32
    wts = {
        "wt_ihc": np.ascontiguousarray(np.asarray(w_ih_c, f32).T),
        "wt_hhc": np.ascontiguousarray(np.asarray(w_hh_c, f32).T),
        "wt_ihp": np.ascontiguousarray(np.asarray(w_ih_p, f32).T),
        "wt_hhp": np.ascontiguousarray(np.asarray(w_hh_p, f32).T),
        "b4_c": _b4(b_ih_c, b_hh_c),
        "b4_p": _b4(b_ih_p, b_hh_p),
    }
    in_maps = []
    for k in range(NCORES):
        s = slice(k * BPC, (k + 1) * BPC)
        m = {
            "adj": np.ascontiguousarray(adj_matrix[s], f32),
            "ps": np.ascontiguousarray(path_states[s], f32),
            "cs": np.ascontiguousarray(channel_states[s], f32),
        }
        m.update(wts)
        in_maps.append(m)
    res = run_bass_kernel_spmd(nc, in_maps, core_ids=list(range(NCORES)),
                               trace=_trace)
    new_path = np.concatenate([res.results[k]["new_path"] for k in range(NCORES)])
    new_channel = np.concatenate(
        [res.results[k]["new_channel"] for k in range(NCORES)])
    out = (new_path, new_channel)
    if _debug or _trace:
        return out, res
    return out
